# revision 1
# baseline (speedup 1.0000x reference)
"""Multi-head attention forward on 8 Trainium2 NeuronCores.

Problem: nn.MultiHeadAttention, input [4, 2048, 1024], 16 heads, head_dim 64.

Sharding: core = (batch b, head-group g) with b = core // 2, g = core % 2.
Each core computes attention for 8 heads (4 head-pairs) of one batch and the
corresponding row-parallel slice of the output projection; the host sums the
two partial outputs per batch and adds the (folded) biases.

On-device layout is "transposed": activations are kept as x^T / Q^T / K^T
[feature, token] so every matmul contracts over the partition dim with N=512
free size, which keeps float32r matmuls at full PE rate.  Scores are computed
transposed (S^T = K @ Q^T) per head with the head-pair row-packed into the
128x128 PE array; softmax denominators ride along as an appended ones-column
in the A@V matmul (M=65); normalization broadcasts 1/denom across partitions
with a mask-row matmul.
"""

import numpy as np

import concourse.bass as bass
import concourse.mybir as mybir
import concourse.tile as tile
from concourse import bacc
from concourse.bass_utils import run_bass_kernel_spmd

B = 4
S = 2048
E = 1024
H = 16
D = 64
N_CORES = 8
HEADS_PER_CORE = H // 2          # 8
PAIRS = HEADS_PER_CORE // 2      # 4
DH = HEADS_PER_CORE * D          # 512, per-core qkv slice width
KT_E = E // 128                  # 8  k-tiles over embed dim
SVT = S // 128                   # 16 skv tiles
CHUNKS = S // 512                # 4  Sq chunks
F32 = mybir.dt.float32
F32R = mybir.dt.float32r

_CACHE = {}

# test.py may set this to capture a profile; the graded path leaves it off.
TRACE = False
LAST_RESULTS = None


def _build_program(repeat=1):
    nc = bacc.Bacc("TRN2", target_bir_lowering=False, debug=False)

    xT_d = nc.dram_tensor("xT", [E, S], F32, kind="ExternalInput")
    wqT_d = nc.dram_tensor("wqT", [E, DH], F32, kind="ExternalInput")
    wkT_d = nc.dram_tensor("wkT", [E, DH], F32, kind="ExternalInput")
    wvT_d = nc.dram_tensor("wvT", [E, DH], F32, kind="ExternalInput")
    woT_d = nc.dram_tensor("woT", [DH, E], F32, kind="ExternalInput")
    bq_d = nc.dram_tensor("bq", [128, PAIRS], F32, kind="ExternalInput")
    bk_d = nc.dram_tensor("bk", [128, PAIRS], F32, kind="ExternalInput")
    mask_d = nc.dram_tensor("mask", [128, 64], F32, kind="ExternalInput")
    ones8_d = nc.dram_tensor("ones8", [128, HEADS_PER_CORE], F32, kind="ExternalInput")
    zeros_d = nc.dram_tensor("zeros", [128, 1024], F32, kind="ExternalInput")
    y_d = nc.dram_tensor("y", [S, E], F32, kind="ExternalOutput")

    EXPF = mybir.ActivationFunctionType.Exp
    MUL = mybir.AluOpType.mult

    with tile.TileContext(nc) as tc:
        with tc.tile_pool(name="persist", bufs=1) as pp:
            # ---- persistent tiles ------------------------------------------------
            mask = pp.tile([128, 64], F32R, name="mask")
            nc.sync.dma_start(mask[:], mask_d.ap().bitcast(F32R))
            bq = pp.tile([128, PAIRS], F32, name="bq")
            bk = pp.tile([128, PAIRS], F32, name="bk")
            nc.sync.dma_start(bq[:], bq_d.ap())
            nc.sync.dma_start(bk[:], bk_d.ap())
            # V' = [V_h | 1] per head, per skv tile
            vp = pp.tile([128, SVT, HEADS_PER_CORE, 65], F32R, name="vp")
            for sv in range(SVT):
                nc.sync.dma_start(vp[:, sv, :, 64], ones8_d.ap().bitcast(F32R))
            attnT = [pp.tile([128, S], F32R, name=f"attnT{p}") for p in range(PAIRS)]
            # 1/denom rows live at partition 64; all other partitions must be
            # zero so the mask-matmul broadcast contracts against clean data.
            recr = pp.tile([128, 1024], F32R, name="recr")
            nc.sync.dma_start(recr[:], zeros_d.ap().bitcast(F32R))

            for rep in range(repeat):
                _emit_iteration(nc, tc, pp, mask, bq, bk, vp, attnT, recr,
                                xT_d, wqT_d, wkT_d, wvT_d, woT_d, y_d, EXPF, MUL)

    nc.compile()
    return nc


def _emit_iteration(nc, tc, pp, mask, bq, bk, vp, attnT, recr,
                    xT_d, wqT_d, wkT_d, wvT_d, woT_d, y_d, EXPF, MUL):
    with (
        tc.tile_pool(name="xtp", bufs=1) as xp,
        tc.tile_pool(name="wqkp", bufs=2 * KT_E) as wqkp,
    ):
        xt = [xp.tile([128, S], F32R, name=f"xt{k}") for k in range(KT_E)]
        for k in range(KT_E):
            nc.sync.dma_start(
                xt[k][:], xT_d.ap()[k * 128:(k + 1) * 128, :].bitcast(F32R))

        with (
            tc.tile_pool(name="psS", bufs=2, space="PSUM") as psS,
            tc.tile_pool(name="psAV", bufs=1, space="PSUM") as psAV,
            tc.tile_pool(name="psP", bufs=1, space="PSUM") as psP,
        ):
            # ---- V projection (all heads at once, N=512) -------------------------
            with tc.tile_pool(name="wvp", bufs=KT_E) as wvp:
                wv = [wvp.tile([128, DH], F32R, name="wv", tag="wv")
                      for _ in range(KT_E)]
                for k in range(KT_E):
                    nc.sync.dma_start(
                        wv[k][:], wvT_d.ap()[k * 128:(k + 1) * 128, :].bitcast(F32R))
                for sv in range(SVT):
                    # ping-pong across the two AV psum banks (idle in this phase)
                    pv = psAV.tile([128, 512], F32, name="pv",
                                   tag=("pae" if sv % 2 == 0 else "pao"))
                    for k in range(KT_E):
                        nc.tensor.matmul(
                            pv[:], xt[k][:, sv * 128:(sv + 1) * 128], wv[k][:],
                            start=(k == 0), stop=(k == KT_E - 1))
                    nc.vector.tensor_copy(
                        vp[:, sv, :, 0:64],
                        pv[:].rearrange("p (h d) -> p h d", h=HEADS_PER_CORE))

            # ---- per-pair projections + attention --------------------------------
            with (
                tc.tile_pool(name="qk", bufs=2) as qkp,
                tc.tile_pool(name="exps", bufs=5) as ep,
                tc.tile_pool(name="norm", bufs=1) as np_,
            ):
              for p in range(PAIRS):
                qt_t = qkp.tile([128, S], F32R, name="qt", tag="qt")
                kt_t = qkp.tile([128, S], F32R, name="kt", tag="kt")
                for dst, w_d_, bias in ((qt_t, wqT_d, bq), (kt_t, wkT_d, bk)):
                    wtiles = []
                    for k in range(KT_E):
                        wt = wqkp.tile([128, 128], F32R, name="wqk", tag="wqk")
                        nc.sync.dma_start(
                            wt[:],
                            w_d_.ap()[k * 128:(k + 1) * 128,
                                      p * 128:(p + 1) * 128].bitcast(F32R))
                        wtiles.append(wt)
                    for c in range(CHUNKS):
                        pq = psP.tile([128, 512], F32, name="pq", tag="pq")
                        for k in range(KT_E):
                            nc.tensor.matmul(
                                pq[:], wtiles[k][:],
                                xt[k][:, c * 512:(c + 1) * 512],
                                start=(k == 0), stop=(k == KT_E - 1))
                        nc.vector.tensor_scalar_add(
                            dst[:, c * 512:(c + 1) * 512], pq[:], bias[:, p:p + 1])

                for c in range(CHUNKS):
                    qe = qt_t[0:64, c * 512:(c + 1) * 512]
                    qo = qt_t[64:128, c * 512:(c + 1) * 512]
                    expE = []
                    expO = []
                    for sg in range(SVT // 2):
                        pse = psS.tile([128, 1024], F32, name="pse", tag="ps")
                        pso = psS.tile([128, 1024], F32, name="pso", tag="ps")
                        for j in range(2):
                            t = sg * 2 + j
                            nc.tensor.matmul(
                                pse[:, j * 512:(j + 1) * 512],
                                kt_t[0:64, t * 128:(t + 1) * 128], qe,
                                start=True, stop=True, tile_position=(0, 0))
                            nc.tensor.matmul(
                                pso[:, j * 512:(j + 1) * 512],
                                kt_t[64:128, t * 128:(t + 1) * 128], qo,
                                start=True, stop=True, tile_position=(64, 0))
                        ee = ep.tile([128, 1024], F32R, name="expE", tag="exp")
                        eo = ep.tile([128, 1024], F32R, name="expO", tag="exp")
                        nc.scalar.activation(ee[:], pse[:], EXPF)
                        nc.scalar.activation(eo[:], pso[:], EXPF)
                        expE.append(ee)
                        expO.append(eo)

                    pae = psAV.tile([128, 512], F32, name="pae", tag="pae")
                    pao = psAV.tile([128, 512], F32, name="pao", tag="pao")
                    for t in range(SVT):
                        sg, j = divmod(t, 2)
                        nc.tensor.matmul(
                            pae[0:65, :], vp[:, t, 2 * p, :],
                            expE[sg][:, j * 512:(j + 1) * 512],
                            start=(t == 0), stop=(t == SVT - 1))
                        nc.tensor.matmul(
                            pao[0:65, :], vp[:, t, 2 * p + 1, :],
                            expO[sg][:, j * 512:(j + 1) * 512],
                            start=(t == 0), stop=(t == SVT - 1))

                    # normalization: 1/denom broadcast via mask-row matmul
                    den = np_.tile([128, 1024], F32, name="den", tag="den")
                    nc.vector.tensor_copy(den[64:65, 0:512], pae[64:65, :])
                    nc.vector.tensor_copy(den[64:65, 512:1024], pao[64:65, :])
                    rec = np_.tile([128, 1024], F32, name="rec", tag="rec")
                    nc.vector.reciprocal(rec[64:65, :], den[64:65, :])
                    nc.vector.tensor_copy(recr[64:65, :], rec[64:65, :].bitcast(F32R))
                    bce = psAV.tile([128, 512], F32, name="bce", tag="bc")
                    bces = np_.tile([128, 512], F32R, name="bces", tag="bces")
                    nc.tensor.matmul(bce[0:64, :], mask[:, :], recr[:, 0:512],
                                     start=True, stop=True)
                    nc.vector.tensor_copy(bces[0:64, :], bce[0:64, :].bitcast(F32R))
                    bco = psAV.tile([128, 512], F32, name="bco", tag="bc")
                    bcos = np_.tile([128, 512], F32R, name="bcos", tag="bcos")
                    nc.tensor.matmul(bco[0:64, :], mask[:, :], recr[:, 512:1024],
                                     start=True, stop=True)
                    nc.vector.tensor_copy(bcos[0:64, :], bco[0:64, :].bitcast(F32R))
                    stg = np_.tile([128, 512], F32R, name="stg", tag="stg")
                    nc.vector.tensor_tensor(
                        attnT[p][0:64, c * 512:(c + 1) * 512],
                        pae[0:64, :], bces[0:64, :], MUL)
                    nc.vector.tensor_tensor(
                        stg[0:64, :], pao[0:64, :], bcos[0:64, :], MUL)
                    nc.sync.dma_start(
                        attnT[p][64:128, c * 512:(c + 1) * 512], stg[0:64, :])

    # ---- output projection ----------------------------------------------------
    with (
        tc.tile_pool(name="wop", bufs=PAIRS) as wop,
        tc.tile_pool(name="ystage", bufs=4) as yp,
        tc.tile_pool(name="psY", bufs=2, space="PSUM") as psY,
    ):
        wo = [wop.tile([128, E], F32R, name="wo", tag="wo") for _ in range(PAIRS)]
        for p in range(PAIRS):
            nc.sync.dma_start(
                wo[p][:], woT_d.ap()[p * 128:(p + 1) * 128, :].bitcast(F32R))
        for tt in range(SVT):
            for nch in range(2):
                py = psY.tile([128, 512], F32, name="py", tag="py")
                for p in range(PAIRS):
                    nc.tensor.matmul(
                        py[:], attnT[p][:, tt * 128:(tt + 1) * 128],
                        wo[p][:, nch * 512:(nch + 1) * 512],
                        start=(p == 0), stop=(p == PAIRS - 1))
                ys = yp.tile([128, 512], F32, name="ys", tag="ys")
                nc.vector.tensor_copy(ys[:], py[:])
                nc.sync.dma_start(
                    y_d.ap()[tt * 128:(tt + 1) * 128, nch * 512:(nch + 1) * 512],
                    ys[:])


def kernel(input_tensor, wq, bq, wk, bk_, wv, bv, wo, bo):
    global LAST_RESULTS
    if "nc" not in _CACHE:
        _CACHE["nc"] = _build_program()
    nc = _CACHE["nc"]

    x = np.asarray(input_tensor, dtype=np.float32)
    scale = np.float32(1.0 / np.sqrt(np.float32(D)))

    wqT = np.ascontiguousarray(np.asarray(wq, np.float32).T * scale)
    wkT = np.ascontiguousarray(np.asarray(wk, np.float32).T)
    wvT = np.ascontiguousarray(np.asarray(wv, np.float32).T)
    woT = np.ascontiguousarray(np.asarray(wo, np.float32).T)
    bqs = np.asarray(bq, np.float32) * scale

    mask = np.zeros((128, 64), np.float32)
    mask[64, :] = 1.0
    ones8 = np.ones((128, HEADS_PER_CORE), np.float32)

    in_maps = []
    for core in range(N_CORES):
        b, g = divmod(core, 2)
        hs = slice(g * DH, (g + 1) * DH)
        in_maps.append({
            "xT": np.ascontiguousarray(x[b].T),
            "wqT": np.ascontiguousarray(wqT[:, hs]),
            "wkT": np.ascontiguousarray(wkT[:, hs]),
            "wvT": np.ascontiguousarray(wvT[:, hs]),
            "woT": np.ascontiguousarray(woT[hs, :]),
            "bq": np.ascontiguousarray(bqs[hs].reshape(PAIRS, 128).T),
            "bk": np.ascontiguousarray(
                np.asarray(bk_, np.float32)[hs].reshape(PAIRS, 128).T),
            "mask": mask,
            "ones8": ones8,
            "zeros": np.zeros((128, 1024), np.float32),
        })

    res = run_bass_kernel_spmd(nc, in_maps, core_ids=list(range(N_CORES)), trace=TRACE)
    LAST_RESULTS = res

    # unshard: sum the two head-group partials per batch, add folded biases
    bias_full = (np.asarray(bo, np.float32)
                 + np.asarray(bv, np.float32) @ woT).astype(np.float32)
    y = np.empty((B, S, E), np.float32)
    for b in range(B):
        y[b] = res.results[2 * b]["y"] + res.results[2 * b + 1]["y"] + bias_full[None, :]
    return y

